# revision 6
# baseline (speedup 1.0000x reference)
"""Bass/Trainium2 kernel for nn_BitwiseTasNetRepeat.

Strategy
--------
Every sign(BN(.)) in the reference collapses to a per-channel threshold
compare (BN gamma > 0), so the whole block chain becomes:

    S1 = sign(R - thr1)                    (ACT Sign, bias = -thr1)
    p1 = sign(w1) @ S1                     (TensorE, bf16 exact: operands +-1)
    S2 = sign(p1 - thr2)                   (ACT Sign from PSUM)
    q  = a0*S2(-d) + S2(0) + a2*S2(+d)     (DVE scalar_tensor_tensor x2)
    S3 = (q >= ctr*thr3) - 0.5             (tensor_scalar is_ge/sub)
    p2 = (sign(w2)*ctr) @ S3               (TensorE)
    R += 2*p2                              (DVE scalar_tensor_tensor)

where d = 2^i, ctr = sign(dw_w[:,1]), a0 = sign(dw_w[:,0])*ctr,
a2 = sign(dw_w[:,2])*ctr.  The center-tap sign ctr is folded into the
conv2 weights and thr3; the 0.5 amplitude of S3 is folded into the
"*2" of the residual update.  All arithmetic is exact in fp32/bf16
(matmul operands are +-1, accumulation in fp32 PSUM).

Sharding: data-parallel over batch, 2 batches per core, 8 cores.
The depthwise stage runs in 1024-column chunks so TensorE keeps
working (conv2 of chunk q overlaps dw of chunk q+1).
"""

import numpy as np
import ml_dtypes

_B, _CB, _H, _T = 16, 256, 512, 4096
_BLOCKS = 8
_EPS = 1e-5
_NCORES = 8
_BS = _B // _NCORES      # batches per core
_KC = _CB // 128         # 2  k-tiles of Cb
_MH = _H // 128          # 4  m-tiles of H
_PAD = 128               # halo for dilated depthwise conv (max d = 128)
_NCC = 10                # f32 const columns per block
_NCB = 8                 # bf16 const columns per block
_QC = 1024               # chunk width for dw/u3/conv2 stages
_NQ = _T // _QC          # 4 chunks

_nc_cache = {}


def _build_nc(bs=_BS, nblocks=_BLOCKS, T=_T):
    import concourse.bass as bass
    import concourse.mybir as mybir
    from concourse import bacc
    from concourse.tile import TileContext

    f32 = mybir.dt.float32
    bf16 = mybir.dt.bfloat16
    ALU = mybir.AluOpType
    nq = T // _QC

    nc = bacc.Bacc("TRN2", target_bir_lowering=False, debug=False,
                   enable_asserts=False)

    x_d = nc.dram_tensor("x", [bs, _CB, T], f32, kind="ExternalInput")
    w1_d = nc.dram_tensor("w1sb", [128, nblocks * _KC * _MH * 128], bf16,
                          kind="ExternalInput")
    w2_d = nc.dram_tensor("w2sb", [128, nblocks * _MH * _KC * 128], bf16,
                          kind="ExternalInput")
    cst_d = nc.dram_tensor("cst", [128, nblocks * _NCC], f32,
                           kind="ExternalInput")
    cstb_d = nc.dram_tensor("cstb", [128, nblocks * _NCB], bf16,
                            kind="ExternalInput")
    out_d = nc.dram_tensor("out", [bs, _CB, T], f32, kind="ExternalOutput")

    with TileContext(nc) as tc:
        with (
            tc.tile_pool(name="wpool", bufs=1) as wpool,
            tc.tile_pool(name="cpool", bufs=1) as cpool,
            tc.tile_pool(name="rpool", bufs=2) as rpool,
            tc.tile_pool(name="s1pool", bufs=3) as s1pool,
            tc.tile_pool(name="s2pool", bufs=5) as s2pool,
            tc.tile_pool(name="s3pool", bufs=18) as s3pool,
            tc.tile_pool(name="qpool", bufs=3) as qpool,
            tc.tile_pool(name="psA", bufs=2, space="PSUM") as psA,
            tc.tile_pool(name="psC", bufs=2, space="PSUM") as psC,
        ):
            w1sb = wpool.tile([128, nblocks * _KC * _MH * 128], bf16)
            nc.sync.dma_start(out=w1sb[:], in_=w1_d.ap())
            w2sb = wpool.tile([128, nblocks * _MH * _KC * 128], bf16)
            nc.sync.dma_start(out=w2sb[:], in_=w2_d.ap())
            cst = cpool.tile([128, nblocks * _NCC], f32)
            nc.sync.dma_start(out=cst[:], in_=cst_d.ap())
            cstb = cpool.tile([128, nblocks * _NCB], bf16)
            nc.sync.dma_start(out=cstb[:], in_=cstb_d.ap())

            def w1t(i, kc, mh):
                o = (i * _KC * _MH + kc * _MH + mh) * 128
                return w1sb[:, o:o + 128]

            def w2t(i, kh, mc):
                o = (i * _MH * _KC + kh * _KC + mc) * 128
                return w2sb[:, o:o + 128]

            def cc(i, j):
                return cst[:, i * _NCC + j:i * _NCC + j + 1]

            def cb(i, j):
                return cstb[:, i * _NCB + j:i * _NCB + j + 1]

            for b in range(bs):
                R = []
                for kc in range(_KC):
                    rt = rpool.tile([128, T], f32, tag="R",
                                    name=f"R_b{b}_{kc}")
                    nc.sync.dma_start(
                        out=rt[:], in_=x_d.ap()[b, kc * 128:(kc + 1) * 128, :])
                    R.append(rt)

                for i in range(nblocks):
                    d = 2 ** i
                    # ---- stage A: u1 threshold + conv1 + u2 threshold ----
                    S1 = []
                    for kc in range(_KC):
                        s1t = s1pool.tile([128, T], bf16, tag="S1",
                                          name=f"S1_b{b}_i{i}_{kc}")
                        nc.scalar.sign(s1t[:], R[kc][:], bias=cc(i, kc))
                        S1.append(s1t)
                    S2 = []
                    for mh in range(_MH):
                        s2t = s2pool.tile([128, T + 2 * _PAD], bf16, tag="S2",
                                          name=f"S2_b{b}_i{i}_{mh}")
                        nc.gpsimd.memset(s2t[:, 0:_PAD], 0.0)
                        nc.gpsimd.memset(s2t[:, _PAD + T:2 * _PAD + T], 0.0)
                        S2.append(s2t)
                    for mh in range(_MH):
                        for g in range(T // 1024):
                            ps = psA.tile([128, 1024], f32, tag="psA",
                                          name=f"psA_{b}_{i}_{mh}_{g}")
                            for nn in range(2):
                                for kc in range(_KC):
                                    nc.tensor.matmul(
                                        ps[:, nn * 512:(nn + 1) * 512],
                                        w1t(i, kc, mh),
                                        S1[kc][:, g * 1024 + nn * 512:
                                               g * 1024 + (nn + 1) * 512],
                                        start=(kc == 0), stop=(kc == _KC - 1))
                            nc.scalar.sign(
                                S2[mh][:, _PAD + g * 1024:_PAD + (g + 1) * 1024],
                                ps[:], bias=cc(i, 2 + mh))
                    # ---- stage B/C interleaved in chunks of _QC columns ----
                    S3 = [[None] * _MH for _ in range(nq)]
                    for q in range(nq):
                        c0 = q * _QC
                        for mh in range(_MH):
                            tmp = qpool.tile([128, _QC], bf16, tag="tmp",
                                             name=f"tmp_b{b}_i{i}_q{q}_{mh}")
                            nc.vector.scalar_tensor_tensor(
                                tmp[:], S2[mh][:, _PAD + c0 - d:_PAD + c0 - d + _QC],
                                cb(i, mh), S2[mh][:, _PAD + c0:_PAD + c0 + _QC],
                                op0=ALU.mult, op1=ALU.add)
                            qt = qpool.tile([128, _QC], bf16, tag="q",
                                            name=f"q_b{b}_i{i}_q{q}_{mh}")
                            nc.vector.scalar_tensor_tensor(
                                qt[:], S2[mh][:, _PAD + c0 + d:_PAD + c0 + d + _QC],
                                cb(i, 4 + mh), tmp[:],
                                op0=ALU.mult, op1=ALU.add)
                            s3t = s3pool.tile([128, _QC], bf16, tag="S3",
                                              name=f"S3_b{b}_i{i}_q{q}_{mh}")
                            nc.vector.tensor_scalar(
                                s3t[:], qt[:], cc(i, 6 + mh), 0.5,
                                op0=ALU.is_ge, op1=ALU.subtract)
                            S3[q][mh] = s3t
                        # conv2 + residual update for this chunk
                        for mc in range(_KC):
                            ps2 = psC.tile([128, _QC], f32, tag="psC",
                                           name=f"psC_{b}_{i}_q{q}_{mc}")
                            for nn in range(2):
                                for kh in range(_MH):
                                    nc.tensor.matmul(
                                        ps2[:, nn * 512:(nn + 1) * 512],
                                        w2t(i, kh, mc),
                                        S3[q][kh][:, nn * 512:(nn + 1) * 512],
                                        start=(kh == 0), stop=(kh == _MH - 1))
                            nc.vector.scalar_tensor_tensor(
                                R[mc][:, c0:c0 + _QC], ps2[:], 2.0,
                                R[mc][:, c0:c0 + _QC],
                                op0=ALU.mult, op1=ALU.add)

                for kc in range(_KC):
                    nc.sync.dma_start(
                        out=out_d.ap()[b, kc * 128:(kc + 1) * 128, :],
                        in_=R[kc][:])
    nc.finalize()
    return nc


def _prep(inputs, nblocks=_BLOCKS):
    """Host-side weight/threshold preprocessing (tiny tensors only)."""
    def thr(g, bb, m, v):
        return (m - bb * np.sqrt(v + _EPS) / g).astype(np.float32)

    w1sb = np.zeros((128, nblocks * _KC * _MH * 128), ml_dtypes.bfloat16)
    w2sb = np.zeros((128, nblocks * _MH * _KC * 128), ml_dtypes.bfloat16)
    cst = np.zeros((128, nblocks * _NCC), np.float32)
    cstb = np.zeros((128, nblocks * _NCB), ml_dtypes.bfloat16)
    for i in range(nblocks):
        t1 = thr(inputs['bn1_gamma'][i], inputs['bn1_beta'][i],
                 inputs['bn1_mean'][i], inputs['bn1_var'][i])          # [Cb]
        t2 = thr(inputs['bn2_gamma'][i], inputs['bn2_beta'][i],
                 inputs['bn2_mean'][i], inputs['bn2_var'][i])          # [H]
        t3 = thr(inputs['bn3_gamma'][i], inputs['bn3_beta'][i],
                 inputs['bn3_mean'][i], inputs['bn3_var'][i])          # [H]
        W1s = np.sign(inputs['w1'][i]).astype(np.float32)              # [H, Cb]
        W2s = np.sign(inputs['w2'][i]).astype(np.float32)              # [Cb, H]
        dws = np.sign(inputs['dw_w'][i]).astype(np.float32)            # [H, 3]
        ctr = dws[:, 1]
        a0 = dws[:, 0] * ctr
        a2 = dws[:, 2] * ctr
        W2p = W2s * ctr[None, :]                                       # [Cb, H]
        for kc in range(_KC):
            for mh in range(_MH):
                o = (i * _KC * _MH + kc * _MH + mh) * 128
                # lhsT1[p, f] = W1s[mh*128+f, kc*128+p]
                w1sb[:, o:o + 128] = W1s[mh * 128:(mh + 1) * 128,
                                         kc * 128:(kc + 1) * 128].T
        for kh in range(_MH):
            for mc in range(_KC):
                o = (i * _MH * _KC + kh * _KC + mc) * 128
                # lhsT2[p, f] = W2p[mc*128+f, kh*128+p]
                w2sb[:, o:o + 128] = W2p[mc * 128:(mc + 1) * 128,
                                         kh * 128:(kh + 1) * 128].T
        base = i * _NCC
        for kc in range(_KC):
            cst[:, base + kc] = -t1[kc * 128:(kc + 1) * 128]
        for mh in range(_MH):
            sl = slice(mh * 128, (mh + 1) * 128)
            cst[:, base + 2 + mh] = -t2[sl]
            cst[:, base + 6 + mh] = (ctr * t3)[sl]
            cstb[:, i * _NCB + mh] = a0[sl]
            cstb[:, i * _NCB + 4 + mh] = a2[sl]
    return w1sb, w2sb, cst, cstb


def kernel(**inputs):
    inputs = {k: np.asarray(v) for k, v in inputs.items()}
    x = inputs['x'].astype(np.float32)
    w1sb, w2sb, cst, cstb = _prep(inputs)

    if 'nc' not in _nc_cache:
        _nc_cache['nc'] = _build_nc()
    nc = _nc_cache['nc']

    in_maps = []
    for c in range(_NCORES):
        in_maps.append({
            'x': np.ascontiguousarray(x[c * _BS:(c + 1) * _BS]),
            'w1sb': w1sb, 'w2sb': w2sb, 'cst': cst, 'cstb': cstb,
        })

    from concourse.bass_utils import run_bass_kernel_spmd
    import os
    trace = bool(int(os.environ.get('KERNEL_TRACE', '0')))
    res = run_bass_kernel_spmd(nc, in_maps, core_ids=list(range(_NCORES)),
                               trace=trace)
    _nc_cache['last_result'] = res
    out = np.concatenate([r['out'] for r in res.results], axis=0)
    return out.astype(np.float32)


# revision 7
# speedup vs baseline: 1.4475x; 1.4475x over previous
"""Bass/Trainium2 kernel for nn_BitwiseTasNetRepeat.

Strategy (v3: all-fp8 with DoubleRow matmuls, depthwise conv on TensorE)
-----------------------------------------------------------------------
Every sign(BN(.)) in the reference collapses to a per-channel threshold
compare (BN gamma > 0), so each block becomes:

    S1 = (R >= t1) - 0.5                    (DVE is_ge, fp8, amp 0.5)
    p1 = sign(w1) @ S1                      (TensorE fp8 DoubleRow, K=256)
    S2 = sign(p1 - 0.5*t2)                  (ACT Sign from PSUM, fp8, amp 1)
    q  = a0*S2(-d) + S2(0) + a2*S2(+d)      (TensorE: DR diag matmul with
                                             overlapping rhs window + plain
                                             diag matmul, accumulated in PSUM)
    S3 = threshold(q vs ctr*t3)             (mh 0,1: ACT Sign amp 1;
                                             mh 2,3: DVE is_ge amp 0.5 --
                                             amplitude folded into w2 cols)
    p2 = w2' @ S3                           (TensorE fp8 DR, K=512 = 2 DR MMs)
    R += p2                                 (DVE tensor_tensor add)

All values are +-1/+-0.5/"small ints", exact in fp8e4m3/fp32-PSUM.
ctr = sign(dw_w[:,1]) is folded into w2 columns and t3; a0/a2 sit in the
diagonal DR weights of the depthwise matmul.

Sharding: data-parallel over batch, 2 batches per core, 8 cores.
Stage B/C run in 1024-column chunks so conv2 of chunk q overlaps the
depthwise matmuls of chunk q+1.
"""

import numpy as np
import ml_dtypes

_B, _CB, _H, _T = 16, 256, 512, 4096
_BLOCKS = 8
_EPS = 1e-5
_NCORES = 8
_BS = _B // _NCORES      # batches per core
_KC = _CB // 128         # 2  k-tiles of Cb
_MH = _H // 128          # 4  m-tiles of H
_PAD = 128               # halo for dilated depthwise conv (max d = 128)
_NCC = 10                # f32 const columns per block
_QC = 1024               # chunk width for dw/u3/conv2 stages

_nc_cache = {}


def _mk3(ap2d, j_step, cols):
    """3D AP [128, 2 (stride j_step), cols] over a 2D row view."""
    import bass_rust
    v = ap2d.copy()
    l = v.ap
    v.ap = bass_rust.VecI64Pair([list(l[0]), [j_step, 2], [1, cols]])
    return v


def _build_nc(bs=_BS, nblocks=_BLOCKS, T=_T):
    import concourse.mybir as mybir
    from concourse import bacc
    from concourse.tile import TileContext

    f32 = mybir.dt.float32
    fp8 = mybir.dt.float8e4
    ALU = mybir.AluOpType
    DRM = mybir.MatmulPerfMode.DoubleRow
    nq = T // _QC

    nc = bacc.Bacc("TRN2", target_bir_lowering=False, debug=False,
                   enable_asserts=False)

    x_d = nc.dram_tensor("x", [bs, _CB, T], f32, kind="ExternalInput")
    w1_d = nc.dram_tensor("w1dr", [128, nblocks * _MH * 256], fp8,
                          kind="ExternalInput")
    w2_d = nc.dram_tensor("w2dr", [128, nblocks * _KC * 2 * 256], fp8,
                          kind="ExternalInput")
    dw01_d = nc.dram_tensor("dw01", [128, nblocks * _MH * 256], fp8,
                            kind="ExternalInput")
    dw2_d = nc.dram_tensor("dw2", [128, nblocks * _MH * 128], fp8,
                           kind="ExternalInput")
    cst_d = nc.dram_tensor("cst", [128, nblocks * _NCC], f32,
                           kind="ExternalInput")
    out_d = nc.dram_tensor("out", [bs, _CB, T], f32, kind="ExternalOutput")

    with TileContext(nc) as tc:
        with (
            tc.tile_pool(name="wpool", bufs=1) as wpool,
            tc.tile_pool(name="rpool", bufs=4) as rpool,
            tc.tile_pool(name="s1pool", bufs=3) as s1pool,
            tc.tile_pool(name="s2pool", bufs=6) as s2pool,
            tc.tile_pool(name="s3pool", bufs=6) as s3pool,
            tc.tile_pool(name="psmm", bufs=2, space="PSUM") as psmm,
            tc.tile_pool(name="psdw", bufs=2, space="PSUM") as psdw,
        ):
            w1sb = wpool.tile([128, nblocks * _MH * 256], fp8)
            nc.sync.dma_start(out=w1sb[:], in_=w1_d.ap())
            w2sb = wpool.tile([128, nblocks * _KC * 2 * 256], fp8)
            nc.sync.dma_start(out=w2sb[:], in_=w2_d.ap())
            dw01sb = wpool.tile([128, nblocks * _MH * 256], fp8)
            nc.sync.dma_start(out=dw01sb[:], in_=dw01_d.ap())
            dw2sb = wpool.tile([128, nblocks * _MH * 128], fp8)
            nc.sync.dma_start(out=dw2sb[:], in_=dw2_d.ap())
            cst = wpool.tile([128, nblocks * _NCC], f32)
            nc.sync.dma_start(out=cst[:], in_=cst_d.ap())

            def w1t(i, mh):
                o = (i * _MH + mh) * 256
                return _mk3(w1sb[:, o:o + 256], 128, 128)

            def w2t(i, mc, pair):
                o = (i * _KC * 2 + mc * 2 + pair) * 256
                return _mk3(w2sb[:, o:o + 256], 128, 128)

            def dw01t(i, mh):
                o = (i * _MH + mh) * 256
                return _mk3(dw01sb[:, o:o + 256], 128, 128)

            def dw2t(i, mh):
                o = (i * _MH + mh) * 128
                return dw2sb[:, o:o + 128]

            def cc(i, j):
                return cst[:, i * _NCC + j:i * _NCC + j + 1]

            for b in range(bs):
                R = []
                for kc in range(_KC):
                    rt = rpool.tile([128, T], f32, tag="R",
                                    name=f"R_b{b}_{kc}")
                    nc.sync.dma_start(
                        out=rt[:], in_=x_d.ap()[b, kc * 128:(kc + 1) * 128, :])
                    R.append(rt)

                for i in range(nblocks):
                    d = 2 ** i
                    # ---- stage A: u1 + conv1 (DR) + u2 ----
                    S1 = s1pool.tile([128, _KC * T], fp8, tag="S1",
                                     name=f"S1_b{b}_i{i}")
                    for kc in range(_KC):
                        nc.vector.tensor_scalar(
                            S1[:, kc * T:(kc + 1) * T], R[kc][:],
                            cc(i, kc), 0.5, op0=ALU.is_ge, op1=ALU.subtract)
                    S2 = []
                    for mh in range(_MH):
                        s2t = s2pool.tile([128, T + 2 * _PAD], fp8, tag="S2",
                                          name=f"S2_b{b}_i{i}_{mh}")
                        nc.gpsimd.memset(s2t[:, 0:_PAD], 0.0)
                        nc.gpsimd.memset(s2t[:, _PAD + T:2 * _PAD + T], 0.0)
                        S2.append(s2t)
                    for mh in range(_MH):
                        for g in range(nq):
                            ps = psmm.tile([128, _QC], f32, tag="mm",
                                           name=f"psA_{b}_{i}_{mh}_{g}")
                            for nn in range(2):
                                c0 = g * _QC + nn * 512
                                rhs = _mk3(S1[:, c0:c0 + 512], T, 512)
                                nc.tensor.matmul(
                                    ps[:, nn * 512:(nn + 1) * 512],
                                    w1t(i, mh), rhs, start=True, stop=True,
                                    perf_mode=DRM)
                            nc.scalar.sign(
                                S2[mh][:, _PAD + g * _QC:_PAD + (g + 1) * _QC],
                                ps[:], bias=cc(i, 2 + mh))
                    # ---- stage B/C in chunks of _QC columns ----
                    for q in range(nq):
                        c0 = q * _QC
                        S3 = [s3pool.tile([128, 2 * _QC], fp8, tag="S3",
                                          name=f"S3_b{b}_i{i}_q{q}_p{p}")
                              for p in range(2)]
                        for mh in range(_MH):
                            pd = psdw.tile([128, _QC], f32, tag="dw",
                                           name=f"psD_{b}_{i}_q{q}_{mh}")
                            for nn in range(2):
                                w0 = _PAD + c0 + nn * 512
                                rhs01 = _mk3(S2[mh][:, w0 - d:w0 - d + 512],
                                             d, 512)
                                nc.tensor.matmul(
                                    pd[:, nn * 512:(nn + 1) * 512],
                                    dw01t(i, mh), rhs01,
                                    start=True, stop=False, perf_mode=DRM)
                                nc.tensor.matmul(
                                    pd[:, nn * 512:(nn + 1) * 512],
                                    dw2t(i, mh),
                                    S2[mh][:, w0 + d:w0 + d + 512],
                                    start=False, stop=True)
                            s3out = S3[mh // 2][:, (mh % 2) * _QC:
                                                (mh % 2 + 1) * _QC]
                            if mh < 2:
                                nc.scalar.sign(s3out, pd[:], bias=cc(i, 6 + mh))
                            else:
                                nc.vector.tensor_scalar(
                                    s3out, pd[:], cc(i, 6 + mh), 0.5,
                                    op0=ALU.is_ge, op1=ALU.subtract)
                        for mc in range(_KC):
                            ps2 = psmm.tile([128, _QC], f32, tag="mm",
                                            name=f"psC_{b}_{i}_q{q}_{mc}")
                            for nn in range(2):
                                for pair in range(2):
                                    rhs = _mk3(S3[pair][:, nn * 512:nn * 512 + 512],
                                               _QC, 512)
                                    nc.tensor.matmul(
                                        ps2[:, nn * 512:(nn + 1) * 512],
                                        w2t(i, mc, pair), rhs,
                                        start=(pair == 0), stop=(pair == 1),
                                        perf_mode=DRM)
                            nc.vector.tensor_tensor(
                                out=R[mc][:, c0:c0 + _QC], in0=ps2[:],
                                in1=R[mc][:, c0:c0 + _QC], op=ALU.add)

                for kc in range(_KC):
                    nc.sync.dma_start(
                        out=out_d.ap()[b, kc * 128:(kc + 1) * 128, :],
                        in_=R[kc][:])
    nc.finalize()
    return nc


def _prep(inputs, nblocks=_BLOCKS):
    """Host-side weight/threshold preprocessing (tiny tensors only)."""
    e4 = ml_dtypes.float8_e4m3

    def thr(g, bb, m, v):
        return (m - bb * np.sqrt(v + _EPS) / g).astype(np.float32)

    w1dr = np.zeros((128, nblocks * _MH * 256), np.float32)
    w2dr = np.zeros((128, nblocks * _KC * 2 * 256), np.float32)
    dw01 = np.zeros((128, nblocks * _MH * 256), np.float32)
    dw2 = np.zeros((128, nblocks * _MH * 128), np.float32)
    cst = np.zeros((128, nblocks * _NCC), np.float32)
    for i in range(nblocks):
        t1 = thr(inputs['bn1_gamma'][i], inputs['bn1_beta'][i],
                 inputs['bn1_mean'][i], inputs['bn1_var'][i])          # [Cb]
        t2 = thr(inputs['bn2_gamma'][i], inputs['bn2_beta'][i],
                 inputs['bn2_mean'][i], inputs['bn2_var'][i])          # [H]
        t3 = thr(inputs['bn3_gamma'][i], inputs['bn3_beta'][i],
                 inputs['bn3_mean'][i], inputs['bn3_var'][i])          # [H]
        W1s = np.sign(inputs['w1'][i]).astype(np.float32)              # [H, Cb]
        W2s = np.sign(inputs['w2'][i]).astype(np.float32)              # [Cb, H]
        dws = np.sign(inputs['dw_w'][i]).astype(np.float32)            # [H, 3]
        ctr = dws[:, 1]
        a0 = dws[:, 0] * ctr
        a2 = dws[:, 2] * ctr
        # fold center-tap sign and the amp-0.5 of the DVE-thresholded
        # channels (kh 2,3) into conv2 weights
        amp = np.where(np.arange(_H) < 256, 1.0, 2.0)
        W2x = W2s * (ctr * amp)[None, :]                               # [Cb, H]
        for mh in range(_MH):
            o = (i * _MH + mh) * 256
            for j in range(2):
                # w1dr[p, j*128+f] = W1s[mh*128+f, j*128+p]
                w1dr[:, o + j * 128:o + (j + 1) * 128] = \
                    W1s[mh * 128:(mh + 1) * 128, j * 128:(j + 1) * 128].T
        for mc in range(_KC):
            for pair in range(2):
                o = (i * _KC * 2 + mc * 2 + pair) * 256
                for j in range(2):
                    kh = pair * 2 + j
                    w2dr[:, o + j * 128:o + (j + 1) * 128] = \
                        W2x[mc * 128:(mc + 1) * 128,
                            kh * 128:(kh + 1) * 128].T
        for mh in range(_MH):
            sl = slice(mh * 128, (mh + 1) * 128)
            o = (i * _MH + mh) * 256
            dw01[np.arange(128), o + np.arange(128)] = a0[sl]
            dw01[np.arange(128), o + 128 + np.arange(128)] = 1.0
            o2 = (i * _MH + mh) * 128
            dw2[np.arange(128), o2 + np.arange(128)] = a2[sl]
        base = i * _NCC
        for kc in range(_KC):
            cst[:, base + kc] = t1[kc * 128:(kc + 1) * 128]
        tau3 = ctr * t3
        for mh in range(_MH):
            sl = slice(mh * 128, (mh + 1) * 128)
            cst[:, base + 2 + mh] = -0.5 * t2[sl]
            # mh 0,1: ACT Sign bias = -tau3 ; mh 2,3: DVE is_ge scalar = tau3
            cst[:, base + 6 + mh] = (-tau3[sl] if mh < 2 else tau3[sl])
    return (w1dr.astype(e4), w2dr.astype(e4), dw01.astype(e4),
            dw2.astype(e4), cst)


def kernel(**inputs):
    inputs = {k: np.asarray(v) for k, v in inputs.items()}
    x = inputs['x'].astype(np.float32)
    w1dr, w2dr, dw01, dw2, cst = _prep(inputs)

    if 'nc' not in _nc_cache:
        _nc_cache['nc'] = _build_nc()
    nc = _nc_cache['nc']

    in_maps = []
    for c in range(_NCORES):
        in_maps.append({
            'x': np.ascontiguousarray(x[c * _BS:(c + 1) * _BS]),
            'w1dr': w1dr, 'w2dr': w2dr, 'dw01': dw01, 'dw2': dw2,
            'cst': cst,
        })

    from concourse.bass_utils import run_bass_kernel_spmd
    import os
    trace = bool(int(os.environ.get('KERNEL_TRACE', '0')))
    res = run_bass_kernel_spmd(nc, in_maps, core_ids=list(range(_NCORES)),
                               trace=trace)
    _nc_cache['last_result'] = res
    out = np.concatenate([r['out'] for r in res.results], axis=0)
    return out.astype(np.float32)


# revision 10
# speedup vs baseline: 1.6520x; 1.1413x over previous
"""Bass/Trainium2 kernel for nn_BitwiseTasNetRepeat.

Strategy (v3: all-fp8 with DoubleRow matmuls, depthwise conv on TensorE)
-----------------------------------------------------------------------
Every sign(BN(.)) in the reference collapses to a per-channel threshold
compare (BN gamma > 0), so each block becomes:

    S1 = (R >= t1) - 0.5                    (DVE is_ge, fp8, amp 0.5)
    p1 = sign(w1) @ S1                      (TensorE fp8 DoubleRow, K=256)
    S2 = sign(p1 - 0.5*t2)                  (ACT Sign from PSUM, fp8, amp 1)
    q  = a0*S2(-d) + S2(0) + a2*S2(+d)      (TensorE: DR diag matmul with
                                             overlapping rhs window + plain
                                             diag matmul, accumulated in PSUM)
    S3 = threshold(q vs ctr*t3)             (mh 0,1: ACT Sign amp 1;
                                             mh 2,3: DVE is_ge amp 0.5 --
                                             amplitude folded into w2 cols)
    p2 = w2' @ S3                           (TensorE fp8 DR, K=512 = 2 DR MMs)
    R += p2                                 (DVE tensor_tensor add)

All values are +-1/+-0.5/"small ints", exact in fp8e4m3/fp32-PSUM.
ctr = sign(dw_w[:,1]) is folded into w2 columns and t3; a0/a2 sit in the
diagonal DR weights of the depthwise matmul.

Sharding: data-parallel over batch, 2 batches per core, 8 cores.
Stage B/C run in 1024-column chunks so conv2 of chunk q overlaps the
depthwise matmuls of chunk q+1.
"""

import numpy as np
import ml_dtypes

_B, _CB, _H, _T = 16, 256, 512, 4096
_BLOCKS = 8
_EPS = 1e-5
_NCORES = 8
_BS = _B // _NCORES      # batches per core
_KC = _CB // 128         # 2  k-tiles of Cb
_MH = _H // 128          # 4  m-tiles of H
_PAD = 128               # halo for dilated depthwise conv (max d = 128)
_NCC = 10                # f32 const columns per block
_QC = 1024               # chunk width for dw/u3/conv2 stages

_nc_cache = {}


def _mk3(ap2d, j_step, cols):
    """3D AP [128, 2 (stride j_step), cols] over a 2D row view."""
    import bass_rust
    v = ap2d.copy()
    l = v.ap
    v.ap = bass_rust.VecI64Pair([list(l[0]), [j_step, 2], [1, cols]])
    return v


def _build_nc(bs=_BS, nblocks=_BLOCKS, T=_T):
    import concourse.mybir as mybir
    from concourse import bacc
    from concourse.tile import TileContext

    f32 = mybir.dt.float32
    fp8 = mybir.dt.float8e4
    ALU = mybir.AluOpType
    DRM = mybir.MatmulPerfMode.DoubleRow
    nq = T // _QC

    nc = bacc.Bacc("TRN2", target_bir_lowering=False, debug=False,
                   enable_asserts=False)

    x_d = nc.dram_tensor("x", [bs, _CB, T], f32, kind="ExternalInput")
    w1_d = nc.dram_tensor("w1dr", [128, nblocks * _MH * 256], fp8,
                          kind="ExternalInput")
    w2_d = nc.dram_tensor("w2dr", [128, nblocks * _KC * 2 * 256], fp8,
                          kind="ExternalInput")
    dw01_d = nc.dram_tensor("dw01", [128, nblocks * _MH * 256], fp8,
                            kind="ExternalInput")
    dw2_d = nc.dram_tensor("dw2", [128, nblocks * _MH * 128], fp8,
                           kind="ExternalInput")
    cst_d = nc.dram_tensor("cst", [128, nblocks * _NCC], f32,
                           kind="ExternalInput")
    out_d = nc.dram_tensor("out", [bs, _CB, T], f32, kind="ExternalOutput")

    with TileContext(nc) as tc:
        with (
            tc.tile_pool(name="wpool", bufs=1) as wpool,
            tc.tile_pool(name="rpool", bufs=4) as rpool,
            tc.tile_pool(name="s1pool", bufs=4) as s1pool,
            tc.tile_pool(name="s2pool", bufs=10) as s2pool,
            tc.tile_pool(name="s3pool", bufs=8) as s3pool,
            tc.tile_pool(name="psmm", bufs=2, space="PSUM") as psmm,
            tc.tile_pool(name="psdw", bufs=2, space="PSUM") as psdw,
        ):
            w1sb = wpool.tile([128, nblocks * _MH * 256], fp8)
            nc.sync.dma_start(out=w1sb[:], in_=w1_d.ap())
            w2sb = wpool.tile([128, nblocks * _KC * 2 * 256], fp8)
            nc.sync.dma_start(out=w2sb[:], in_=w2_d.ap())
            dw01sb = wpool.tile([128, nblocks * _MH * 256], fp8)
            nc.sync.dma_start(out=dw01sb[:], in_=dw01_d.ap())
            dw2sb = wpool.tile([128, nblocks * _MH * 128], fp8)
            nc.sync.dma_start(out=dw2sb[:], in_=dw2_d.ap())
            cst = wpool.tile([128, nblocks * _NCC], f32)
            nc.sync.dma_start(out=cst[:], in_=cst_d.ap())

            def w1t(i, mh):
                o = (i * _MH + mh) * 256
                return _mk3(w1sb[:, o:o + 256], 128, 128)

            def w2t(i, mc, pair):
                o = (i * _KC * 2 + mc * 2 + pair) * 256
                return _mk3(w2sb[:, o:o + 256], 128, 128)

            def dw01t(i, mh):
                o = (i * _MH + mh) * 256
                return _mk3(dw01sb[:, o:o + 256], 128, 128)

            def dw2t(i, mh):
                o = (i * _MH + mh) * 128
                return dw2sb[:, o:o + 128]

            def cc(i, j):
                return cst[:, i * _NCC + j:i * _NCC + j + 1]

            Rb = {}
            for b in range(bs):
                Rb[b] = []
                for kc in range(_KC):
                    rt = rpool.tile([128, T], f32, tag="R",
                                    name=f"R_b{b}_{kc}")
                    nc.sync.dma_start(
                        out=rt[:], in_=x_d.ap()[b, kc * 128:(kc + 1) * 128, :])
                    Rb[b].append(rt)

            for i in range(nblocks):
                for b in range(bs):
                    R = Rb[b]
                    d = 2 ** i
                    # ---- stage A: u1 + conv1 (DR) + u2 ----
                    S1 = s1pool.tile([128, _KC * T], fp8, tag="S1",
                                     name=f"S1_b{b}_i{i}")
                    for kc in range(_KC):
                        nc.vector.tensor_scalar(
                            S1[:, kc * T:(kc + 1) * T], R[kc][:],
                            cc(i, kc), 0.5, op0=ALU.is_ge, op1=ALU.subtract)
                    S2 = []
                    for mh in range(_MH):
                        s2t = s2pool.tile([128, T + 2 * _PAD], fp8, tag="S2",
                                          name=f"S2_b{b}_i{i}_{mh}")
                        nc.gpsimd.memset(s2t[:, 0:_PAD], 0.0)
                        nc.gpsimd.memset(s2t[:, _PAD + T:2 * _PAD + T], 0.0)
                        S2.append(s2t)
                    for mh in range(_MH):
                        for g in range(nq):
                            ps = psmm.tile([128, _QC], f32, tag="mm",
                                           name=f"psA_{b}_{i}_{mh}_{g}")
                            for nn in range(2):
                                c0 = g * _QC + nn * 512
                                rhs = _mk3(S1[:, c0:c0 + 512], T, 512)
                                nc.tensor.matmul(
                                    ps[:, nn * 512:(nn + 1) * 512],
                                    w1t(i, mh), rhs, start=True, stop=True,
                                    perf_mode=DRM)
                            nc.scalar.sign(
                                S2[mh][:, _PAD + g * _QC:_PAD + (g + 1) * _QC],
                                ps[:], bias=cc(i, 2 + mh))
                    # ---- stage B/C in chunks of _QC columns ----
                    for q in range(nq):
                        c0 = q * _QC
                        S3 = [s3pool.tile([128, 2 * _QC], fp8, tag="S3",
                                          name=f"S3_b{b}_i{i}_q{q}_p{p}")
                              for p in range(2)]
                        for mh in range(_MH):
                            pd = psdw.tile([128, _QC], f32, tag="dw",
                                           name=f"psD_{b}_{i}_q{q}_{mh}")
                            for nn in range(2):
                                w0 = _PAD + c0 + nn * 512
                                rhs01 = _mk3(S2[mh][:, w0 - d:w0 - d + 512],
                                             d, 512)
                                nc.tensor.matmul(
                                    pd[:, nn * 512:(nn + 1) * 512],
                                    dw01t(i, mh), rhs01,
                                    start=True, stop=False, perf_mode=DRM)
                                nc.tensor.matmul(
                                    pd[:, nn * 512:(nn + 1) * 512],
                                    dw2t(i, mh),
                                    S2[mh][:, w0 + d:w0 + d + 512],
                                    start=False, stop=True)
                            s3out = S3[mh // 2][:, (mh % 2) * _QC:
                                                (mh % 2 + 1) * _QC]
                            if mh < 2:
                                nc.scalar.sign(s3out, pd[:], bias=cc(i, 6 + mh))
                            else:
                                nc.vector.tensor_scalar(
                                    s3out, pd[:], cc(i, 6 + mh), 0.5,
                                    op0=ALU.is_ge, op1=ALU.subtract)
                        for mc in range(_KC):
                            ps2 = psmm.tile([128, _QC], f32, tag="mm",
                                            name=f"psC_{b}_{i}_q{q}_{mc}")
                            for nn in range(2):
                                for pair in range(2):
                                    rhs = _mk3(S3[pair][:, nn * 512:nn * 512 + 512],
                                               _QC, 512)
                                    nc.tensor.matmul(
                                        ps2[:, nn * 512:(nn + 1) * 512],
                                        w2t(i, mc, pair), rhs,
                                        start=(pair == 0), stop=(pair == 1),
                                        perf_mode=DRM)
                            nc.vector.tensor_tensor(
                                out=R[mc][:, c0:c0 + _QC], in0=ps2[:],
                                in1=R[mc][:, c0:c0 + _QC], op=ALU.add)

            for b in range(bs):
                for kc in range(_KC):
                    nc.sync.dma_start(
                        out=out_d.ap()[b, kc * 128:(kc + 1) * 128, :],
                        in_=Rb[b][kc][:])
    nc.finalize()
    return nc


def _prep(inputs, nblocks=_BLOCKS):
    """Host-side weight/threshold preprocessing (tiny tensors only)."""
    e4 = ml_dtypes.float8_e4m3

    def thr(g, bb, m, v):
        return (m - bb * np.sqrt(v + _EPS) / g).astype(np.float32)

    w1dr = np.zeros((128, nblocks * _MH * 256), np.float32)
    w2dr = np.zeros((128, nblocks * _KC * 2 * 256), np.float32)
    dw01 = np.zeros((128, nblocks * _MH * 256), np.float32)
    dw2 = np.zeros((128, nblocks * _MH * 128), np.float32)
    cst = np.zeros((128, nblocks * _NCC), np.float32)
    for i in range(nblocks):
        t1 = thr(inputs['bn1_gamma'][i], inputs['bn1_beta'][i],
                 inputs['bn1_mean'][i], inputs['bn1_var'][i])          # [Cb]
        t2 = thr(inputs['bn2_gamma'][i], inputs['bn2_beta'][i],
                 inputs['bn2_mean'][i], inputs['bn2_var'][i])          # [H]
        t3 = thr(inputs['bn3_gamma'][i], inputs['bn3_beta'][i],
                 inputs['bn3_mean'][i], inputs['bn3_var'][i])          # [H]
        W1s = np.sign(inputs['w1'][i]).astype(np.float32)              # [H, Cb]
        W2s = np.sign(inputs['w2'][i]).astype(np.float32)              # [Cb, H]
        dws = np.sign(inputs['dw_w'][i]).astype(np.float32)            # [H, 3]
        ctr = dws[:, 1]
        a0 = dws[:, 0] * ctr
        a2 = dws[:, 2] * ctr
        # fold center-tap sign and the amp-0.5 of the DVE-thresholded
        # channels (kh 2,3) into conv2 weights
        amp = np.where(np.arange(_H) < 256, 1.0, 2.0)
        W2x = W2s * (ctr * amp)[None, :]                               # [Cb, H]
        for mh in range(_MH):
            o = (i * _MH + mh) * 256
            for j in range(2):
                # w1dr[p, j*128+f] = W1s[mh*128+f, j*128+p]
                w1dr[:, o + j * 128:o + (j + 1) * 128] = \
                    W1s[mh * 128:(mh + 1) * 128, j * 128:(j + 1) * 128].T
        for mc in range(_KC):
            for pair in range(2):
                o = (i * _KC * 2 + mc * 2 + pair) * 256
                for j in range(2):
                    kh = pair * 2 + j
                    w2dr[:, o + j * 128:o + (j + 1) * 128] = \
                        W2x[mc * 128:(mc + 1) * 128,
                            kh * 128:(kh + 1) * 128].T
        for mh in range(_MH):
            sl = slice(mh * 128, (mh + 1) * 128)
            o = (i * _MH + mh) * 256
            dw01[np.arange(128), o + np.arange(128)] = a0[sl]
            dw01[np.arange(128), o + 128 + np.arange(128)] = 1.0
            o2 = (i * _MH + mh) * 128
            dw2[np.arange(128), o2 + np.arange(128)] = a2[sl]
        base = i * _NCC
        for kc in range(_KC):
            cst[:, base + kc] = t1[kc * 128:(kc + 1) * 128]
        tau3 = ctr * t3
        for mh in range(_MH):
            sl = slice(mh * 128, (mh + 1) * 128)
            cst[:, base + 2 + mh] = -0.5 * t2[sl]
            # mh 0,1: ACT Sign bias = -tau3 ; mh 2,3: DVE is_ge scalar = tau3
            cst[:, base + 6 + mh] = (-tau3[sl] if mh < 2 else tau3[sl])
    return (w1dr.astype(e4), w2dr.astype(e4), dw01.astype(e4),
            dw2.astype(e4), cst)


def kernel(**inputs):
    inputs = {k: np.asarray(v) for k, v in inputs.items()}
    x = inputs['x'].astype(np.float32)
    w1dr, w2dr, dw01, dw2, cst = _prep(inputs)

    if 'nc' not in _nc_cache:
        _nc_cache['nc'] = _build_nc()
    nc = _nc_cache['nc']

    in_maps = []
    for c in range(_NCORES):
        in_maps.append({
            'x': np.ascontiguousarray(x[c * _BS:(c + 1) * _BS]),
            'w1dr': w1dr, 'w2dr': w2dr, 'dw01': dw01, 'dw2': dw2,
            'cst': cst,
        })

    from concourse.bass_utils import run_bass_kernel_spmd
    import os
    trace = bool(int(os.environ.get('KERNEL_TRACE', '0')))
    res = run_bass_kernel_spmd(nc, in_maps, core_ids=list(range(_NCORES)),
                               trace=trace)
    _nc_cache['last_result'] = res
    out = np.concatenate([r['out'] for r in res.results], axis=0)
    return out.astype(np.float32)


# revision 11
# speedup vs baseline: 1.9123x; 1.1576x over previous
"""Bass/Trainium2 kernel for nn_BitwiseTasNetRepeat.

Strategy (v3: all-fp8 with DoubleRow matmuls, depthwise conv on TensorE)
-----------------------------------------------------------------------
Every sign(BN(.)) in the reference collapses to a per-channel threshold
compare (BN gamma > 0), so each block becomes:

    S1 = (R >= t1) - 0.5                    (DVE is_ge, fp8, amp 0.5)
    p1 = sign(w1) @ S1                      (TensorE fp8 DoubleRow, K=256)
    S2 = sign(p1 - 0.5*t2)                  (ACT Sign from PSUM, fp8, amp 1)
    q  = a0*S2(-d) + S2(0) + a2*S2(+d)      (TensorE: DR diag matmul with
                                             overlapping rhs window + plain
                                             diag matmul, accumulated in PSUM)
    S3 = threshold(q vs ctr*t3)             (mh 0,1: ACT Sign amp 1;
                                             mh 2,3: DVE is_ge amp 0.5 --
                                             amplitude folded into w2 cols)
    p2 = w2' @ S3                           (TensorE fp8 DR, K=512 = 2 DR MMs)
    R += p2                                 (DVE tensor_tensor add)

All values are +-1/+-0.5/"small ints", exact in fp8e4m3/fp32-PSUM.
ctr = sign(dw_w[:,1]) is folded into w2 columns and t3; a0/a2 sit in the
diagonal DR weights of the depthwise matmul.

Sharding: data-parallel over batch, 2 batches per core, 8 cores.
Stage B/C run in 1024-column chunks so conv2 of chunk q overlaps the
depthwise matmuls of chunk q+1.
"""

import numpy as np
import ml_dtypes

_B, _CB, _H, _T = 16, 256, 512, 4096
_BLOCKS = 8
_EPS = 1e-5
_NCORES = 8
_BS = _B // _NCORES      # batches per core
_KC = _CB // 128         # 2  k-tiles of Cb
_MH = _H // 128          # 4  m-tiles of H
_PAD = 128               # halo for dilated depthwise conv (max d = 128)
_NCC = 10                # f32 const columns per block
_QC = 1024               # chunk width for dw/u3/conv2 stages

_nc_cache = {}


def _mk3(ap2d, j_step, cols):
    """3D AP [128, 2 (stride j_step), cols] over a 2D row view."""
    import bass_rust
    v = ap2d.copy()
    l = v.ap
    v.ap = bass_rust.VecI64Pair([list(l[0]), [j_step, 2], [1, cols]])
    return v


def _build_nc(bs=_BS, nblocks=_BLOCKS, T=_T):
    import concourse.mybir as mybir
    from concourse import bacc
    from concourse.tile import TileContext

    f32 = mybir.dt.float32
    fp8 = mybir.dt.float8e4
    ALU = mybir.AluOpType
    DRM = mybir.MatmulPerfMode.DoubleRow
    nq = T // _QC

    nc = bacc.Bacc("TRN2", target_bir_lowering=False, debug=False,
                   enable_asserts=False)

    x_d = nc.dram_tensor("x", [bs, _CB, T], f32, kind="ExternalInput")
    w1_d = nc.dram_tensor("w1dr", [128, nblocks * _MH * 256], fp8,
                          kind="ExternalInput")
    w2_d = nc.dram_tensor("w2dr", [128, nblocks * _KC * 2 * 256], fp8,
                          kind="ExternalInput")
    dw01_d = nc.dram_tensor("dw01", [128, nblocks * _MH * 256], fp8,
                            kind="ExternalInput")
    dw2_d = nc.dram_tensor("dw2", [128, nblocks * _MH * 128], fp8,
                           kind="ExternalInput")
    cst_d = nc.dram_tensor("cst", [128, nblocks * _NCC], f32,
                           kind="ExternalInput")
    out_d = nc.dram_tensor("out", [bs, _CB, T], f32, kind="ExternalOutput")

    with TileContext(nc) as tc:
        with (
            tc.tile_pool(name="wpool", bufs=1) as wpool,
            tc.tile_pool(name="rpool", bufs=4) as rpool,
            tc.tile_pool(name="s1pool", bufs=4) as s1pool,
            tc.tile_pool(name="s2pool", bufs=10) as s2pool,
            tc.tile_pool(name="s3pool", bufs=8) as s3pool,
            tc.tile_pool(name="psmm", bufs=2, space="PSUM") as psmm,
            tc.tile_pool(name="psdw", bufs=2, space="PSUM") as psdw,
        ):
            w1sb = wpool.tile([128, nblocks * _MH * 256], fp8)
            nc.sync.dma_start(out=w1sb[:], in_=w1_d.ap())
            w2sb = wpool.tile([128, nblocks * _KC * 2 * 256], fp8)
            nc.sync.dma_start(out=w2sb[:], in_=w2_d.ap())
            dw01sb = wpool.tile([128, nblocks * _MH * 256], fp8)
            nc.sync.dma_start(out=dw01sb[:], in_=dw01_d.ap())
            dw2sb = wpool.tile([128, nblocks * _MH * 128], fp8)
            nc.sync.dma_start(out=dw2sb[:], in_=dw2_d.ap())
            cst = wpool.tile([128, nblocks * _NCC], f32)
            nc.sync.dma_start(out=cst[:], in_=cst_d.ap())

            def w1t(i, mh):
                o = (i * _MH + mh) * 256
                return _mk3(w1sb[:, o:o + 256], 128, 128)

            def w2t(i, mc, pair):
                o = (i * _KC * 2 + mc * 2 + pair) * 256
                return _mk3(w2sb[:, o:o + 256], 128, 128)

            def dw01t(i, mh):
                o = (i * _MH + mh) * 256
                return _mk3(dw01sb[:, o:o + 256], 128, 128)

            def dw2t(i, mh):
                o = (i * _MH + mh) * 128
                return dw2sb[:, o:o + 128]

            def cc(i, j):
                return cst[:, i * _NCC + j:i * _NCC + j + 1]

            Rb = {}
            for b in range(bs):
                Rb[b] = []
                for kc in range(_KC):
                    rt = rpool.tile([128, T], f32, tag="R",
                                    name=f"R_b{b}_{kc}")
                    nc.sync.dma_start(
                        out=rt[:], in_=x_d.ap()[b, kc * 128:(kc + 1) * 128, :])
                    Rb[b].append(rt)

            state = {}

            def emitA_u1(b, i):
                R = Rb[b]
                S1 = s1pool.tile([128, _KC * T], fp8, tag="S1",
                                 name=f"S1_b{b}_i{i}")
                for kc in range(_KC):
                    nc.vector.tensor_scalar(
                        S1[:, kc * T:(kc + 1) * T], R[kc][:],
                        cc(i, kc), 0.5, op0=ALU.is_ge, op1=ALU.subtract)
                S2 = []
                for mh in range(_MH):
                    s2t = s2pool.tile([128, T + 2 * _PAD], fp8, tag="S2",
                                      name=f"S2_b{b}_i{i}_{mh}")
                    nc.gpsimd.memset(s2t[:, 0:_PAD], 0.0)
                    nc.gpsimd.memset(s2t[:, _PAD + T:2 * _PAD + T], 0.0)
                    S2.append(s2t)
                state[(b, i)] = (S1, S2)

            def emitA_mh(b, i, mh):
                S1, S2 = state[(b, i)]
                for g in range(nq):
                    ps = psmm.tile([128, _QC], f32, tag="mm",
                                   name=f"psA_{b}_{i}_{mh}_{g}")
                    for nn in range(2):
                        c0 = g * _QC + nn * 512
                        rhs = _mk3(S1[:, c0:c0 + 512], T, 512)
                        nc.tensor.matmul(
                            ps[:, nn * 512:(nn + 1) * 512],
                            w1t(i, mh), rhs, start=True, stop=True,
                            perf_mode=DRM)
                    nc.scalar.sign(
                        S2[mh][:, _PAD + g * _QC:_PAD + (g + 1) * _QC],
                        ps[:], bias=cc(i, 2 + mh))

            def emitBC_q(b, i, q):
                d = 2 ** i
                R = Rb[b]
                _, S2 = state[(b, i)]
                c0 = q * _QC
                S3 = [s3pool.tile([128, 2 * _QC], fp8, tag="S3",
                                  name=f"S3_b{b}_i{i}_q{q}_p{p}")
                      for p in range(2)]
                for mh in range(_MH):
                    pd = psdw.tile([128, _QC], f32, tag="dw",
                                   name=f"psD_{b}_{i}_q{q}_{mh}")
                    for nn in range(2):
                        w0 = _PAD + c0 + nn * 512
                        rhs01 = _mk3(S2[mh][:, w0 - d:w0 - d + 512], d, 512)
                        nc.tensor.matmul(
                            pd[:, nn * 512:(nn + 1) * 512],
                            dw01t(i, mh), rhs01,
                            start=True, stop=False, perf_mode=DRM)
                        nc.tensor.matmul(
                            pd[:, nn * 512:(nn + 1) * 512],
                            dw2t(i, mh),
                            S2[mh][:, w0 + d:w0 + d + 512],
                            start=False, stop=True)
                    s3out = S3[mh // 2][:, (mh % 2) * _QC:(mh % 2 + 1) * _QC]
                    if mh < 2:
                        nc.scalar.sign(s3out, pd[:], bias=cc(i, 6 + mh))
                    else:
                        nc.vector.tensor_scalar(
                            s3out, pd[:], cc(i, 6 + mh), 0.5,
                            op0=ALU.is_ge, op1=ALU.subtract)
                for mc in range(_KC):
                    ps2 = psmm.tile([128, _QC], f32, tag="mm",
                                    name=f"psC_{b}_{i}_q{q}_{mc}")
                    for nn in range(2):
                        for pair in range(2):
                            rhs = _mk3(S3[pair][:, nn * 512:nn * 512 + 512],
                                       _QC, 512)
                            nc.tensor.matmul(
                                ps2[:, nn * 512:(nn + 1) * 512],
                                w2t(i, mc, pair), rhs,
                                start=(pair == 0), stop=(pair == 1),
                                perf_mode=DRM)
                    nc.vector.tensor_tensor(
                        out=R[mc][:, c0:c0 + _QC], in0=ps2[:],
                        in1=R[mc][:, c0:c0 + _QC], op=ALU.add)

            # software-pipelined emission: stage A of step k+1 interleaves
            # with stage B/C of step k so PE demand on PSUM stays smooth
            seq = [(b, i) for i in range(nblocks) for b in range(bs)]
            emitA_u1(*seq[0])
            for j in range(_MH):
                emitA_mh(*seq[0], j)
            for k in range(len(seq)):
                if k + 1 < len(seq):
                    emitA_u1(*seq[k + 1])
                for j in range(_MH):
                    if k + 1 < len(seq):
                        emitA_mh(*seq[k + 1], j)
                    emitBC_q(*seq[k], j)
                state.pop(seq[k])

            for b in range(bs):
                for kc in range(_KC):
                    nc.sync.dma_start(
                        out=out_d.ap()[b, kc * 128:(kc + 1) * 128, :],
                        in_=Rb[b][kc][:])
    nc.finalize()
    return nc


def _prep(inputs, nblocks=_BLOCKS):
    """Host-side weight/threshold preprocessing (tiny tensors only)."""
    e4 = ml_dtypes.float8_e4m3

    def thr(g, bb, m, v):
        return (m - bb * np.sqrt(v + _EPS) / g).astype(np.float32)

    w1dr = np.zeros((128, nblocks * _MH * 256), np.float32)
    w2dr = np.zeros((128, nblocks * _KC * 2 * 256), np.float32)
    dw01 = np.zeros((128, nblocks * _MH * 256), np.float32)
    dw2 = np.zeros((128, nblocks * _MH * 128), np.float32)
    cst = np.zeros((128, nblocks * _NCC), np.float32)
    for i in range(nblocks):
        t1 = thr(inputs['bn1_gamma'][i], inputs['bn1_beta'][i],
                 inputs['bn1_mean'][i], inputs['bn1_var'][i])          # [Cb]
        t2 = thr(inputs['bn2_gamma'][i], inputs['bn2_beta'][i],
                 inputs['bn2_mean'][i], inputs['bn2_var'][i])          # [H]
        t3 = thr(inputs['bn3_gamma'][i], inputs['bn3_beta'][i],
                 inputs['bn3_mean'][i], inputs['bn3_var'][i])          # [H]
        W1s = np.sign(inputs['w1'][i]).astype(np.float32)              # [H, Cb]
        W2s = np.sign(inputs['w2'][i]).astype(np.float32)              # [Cb, H]
        dws = np.sign(inputs['dw_w'][i]).astype(np.float32)            # [H, 3]
        ctr = dws[:, 1]
        a0 = dws[:, 0] * ctr
        a2 = dws[:, 2] * ctr
        # fold center-tap sign and the amp-0.5 of the DVE-thresholded
        # channels (kh 2,3) into conv2 weights
        amp = np.where(np.arange(_H) < 256, 1.0, 2.0)
        W2x = W2s * (ctr * amp)[None, :]                               # [Cb, H]
        for mh in range(_MH):
            o = (i * _MH + mh) * 256
            for j in range(2):
                # w1dr[p, j*128+f] = W1s[mh*128+f, j*128+p]
                w1dr[:, o + j * 128:o + (j + 1) * 128] = \
                    W1s[mh * 128:(mh + 1) * 128, j * 128:(j + 1) * 128].T
        for mc in range(_KC):
            for pair in range(2):
                o = (i * _KC * 2 + mc * 2 + pair) * 256
                for j in range(2):
                    kh = pair * 2 + j
                    w2dr[:, o + j * 128:o + (j + 1) * 128] = \
                        W2x[mc * 128:(mc + 1) * 128,
                            kh * 128:(kh + 1) * 128].T
        for mh in range(_MH):
            sl = slice(mh * 128, (mh + 1) * 128)
            o = (i * _MH + mh) * 256
            dw01[np.arange(128), o + np.arange(128)] = a0[sl]
            dw01[np.arange(128), o + 128 + np.arange(128)] = 1.0
            o2 = (i * _MH + mh) * 128
            dw2[np.arange(128), o2 + np.arange(128)] = a2[sl]
        base = i * _NCC
        for kc in range(_KC):
            cst[:, base + kc] = t1[kc * 128:(kc + 1) * 128]
        tau3 = ctr * t3
        for mh in range(_MH):
            sl = slice(mh * 128, (mh + 1) * 128)
            cst[:, base + 2 + mh] = -0.5 * t2[sl]
            # mh 0,1: ACT Sign bias = -tau3 ; mh 2,3: DVE is_ge scalar = tau3
            cst[:, base + 6 + mh] = (-tau3[sl] if mh < 2 else tau3[sl])
    return (w1dr.astype(e4), w2dr.astype(e4), dw01.astype(e4),
            dw2.astype(e4), cst)


def kernel(**inputs):
    inputs = {k: np.asarray(v) for k, v in inputs.items()}
    x = inputs['x'].astype(np.float32)
    w1dr, w2dr, dw01, dw2, cst = _prep(inputs)

    if 'nc' not in _nc_cache:
        _nc_cache['nc'] = _build_nc()
    nc = _nc_cache['nc']

    in_maps = []
    for c in range(_NCORES):
        in_maps.append({
            'x': np.ascontiguousarray(x[c * _BS:(c + 1) * _BS]),
            'w1dr': w1dr, 'w2dr': w2dr, 'dw01': dw01, 'dw2': dw2,
            'cst': cst,
        })

    from concourse.bass_utils import run_bass_kernel_spmd
    import os
    trace = bool(int(os.environ.get('KERNEL_TRACE', '0')))
    res = run_bass_kernel_spmd(nc, in_maps, core_ids=list(range(_NCORES)),
                               trace=trace)
    _nc_cache['last_result'] = res
    out = np.concatenate([r['out'] for r in res.results], axis=0)
    return out.astype(np.float32)
